# revision 6
# baseline (speedup 1.0000x reference)
"""Chamfer distance (nn_L1_ChamferEval) Trainium2 Bass kernel.

Full inputs: array1 [16, 4096, 3] f32, array2 [16, 4096, 3] f32.
Output: scalar f32 = (mean(sqrt(min_m d)) + mean(sqrt(min_n d))) / 2 * 1000.

Strategy
--------
Data-parallel over batch: core j handles batches {2j, 2j+1}.

d[n,m] = ||x_n||^2 + ||y_m||^2 - 2<x_n,y_m> is computed on the tensor
engine as a single K=13 matmul using augmented rows, with fp32 values
split into bf16 hi/lo pairs so the bf16-rate PE path keeps ~fp32-product
precision (dropped lo*lo terms ~2^-18 relative):

  stationary rows: [xh0..2, xh0..2, xl0..2, nx_h, nx_l, 1, 1]
  moving rows:     [-2yh0..2, -2yl0..2, -2yh0..2, 1, 1, ny_h, ny_l]

K=13 < 32, so 16 independent 32x32 PE sub-tiles run concurrently via
tile_position (inferred from operand base partitions): row-strip r holds a
replica of the operands at partitions 32r, col-strip c produces n-block c.
One "burst" = 16 matmuls filling a [128 n, 2048 m] PSUM block (4 banks).

The min-reduction drains PSUM through both the DVE and ACT ports:
ACT copies the upper 1024 columns to SBUF while DVE runs a single
tensor_tensor_reduce(min, min) consuming (lower 1024 PSUM, ACT copy)
pairs — 2 values/cycle/lane with a chained per-partition running min.

Per-core device output: the 128 x (2 batch * 2 dir * 32 tiles) matrix of
min distances. Host finishes with relu/sqrt/mean in float64 (ordering of
the 131072 min values is irrelevant to the final mean).
"""
import sys
import numpy as np

sys.path.insert(0, "/opt/trn_rl_repo")

import ml_dtypes

N_CORES = 8
B, N, C = 16, 4096, 3
BPC = B // N_CORES          # batches per core
K = 18                      # augmented contraction rows
NT = N // 128               # 32 n-tiles per batch-direction
BF = ml_dtypes.bfloat16

_CACHE = {}


def _build(repeat=1):
    """Build + compile the 8-core bass program. Cached per `repeat`."""
    if repeat in _CACHE:
        return _CACHE[repeat]
    import concourse.bacc as bacc
    import concourse.tile as tile
    from concourse import mybir

    f32 = mybir.dt.float32
    bf16 = mybir.dt.bfloat16
    MIN = mybir.AluOpType.min

    nc = bacc.Bacc("TRN2", target_bir_lowering=False, debug=False,
                   enable_asserts=True, num_devices=N_CORES)

    xs_d = nc.dram_tensor("xs", [BPC, K, N], bf16, kind="ExternalInput").ap()
    ym_d = nc.dram_tensor("ym", [BPC, K, N], bf16, kind="ExternalInput").ap()
    ys_d = nc.dram_tensor("ys", [BPC, K, N], bf16, kind="ExternalInput").ap()
    xm_d = nc.dram_tensor("xm", [BPC, K, N], bf16, kind="ExternalInput").ap()
    mins_d = nc.dram_tensor("mins", [128, 4 * NT], f32,
                            kind="ExternalOutput").ap()

    with tile.TileContext(nc) as tc:
        with (
            tc.tile_pool(name="forms", bufs=1) as formp,
            tc.tile_pool(name="minb", bufs=1) as minbp,
            tc.tile_pool(name="psum", bufs=2, space="PSUM") as psp,
        ):
            # Load the 4 operand forms x 2 batches, replicated on the 4
            # 32-partition row strips the PE sub-tiles read from.
            sb = {}
            for name, dram in (("xs", xs_d), ("ym", ym_d),
                               ("ys", ys_d), ("xm", xm_d)):
                for b in range(BPC):
                    t = formp.tile([128, N], bf16, tag=f"{name}{b}")
                    for s in range(4):
                        nc.sync.dma_start(out=t[32 * s:32 * s + K, :],
                                          in_=dram[b])
                    sb[(name, b)] = t

            minb2 = minbp.tile([128, 4 * NT, 2], f32)
            minbuf = minbp.tile([128, 4 * NT], f32)

            def body():
                col = 0
                for b in range(BPC):
                    for d in range(2):
                        stat = sb[("xs", b)] if d == 0 else sb[("ys", b)]
                        mov = sb[("ym", b)] if d == 0 else sb[("xm", b)]
                        for t in range(NT):
                            for g in range(2):
                                ps = psp.tile([128, 2048], f32)
                                for r in range(4):
                                    for c4 in range(4):
                                        nc.tensor.matmul(
                                            out=ps[32 * c4:32 * c4 + 32,
                                                   512 * r:512 * (r + 1)],
                                            lhsT=stat[32 * r:32 * r + K,
                                                      128 * t + 32 * c4:
                                                      128 * t + 32 * c4 + 32],
                                            rhs=mov[32 * r:32 * r + K,
                                                    2048 * g + 512 * r:
                                                    2048 * g + 512 * (r + 1)],
                                            start=True, stop=True,
                                            tile_position=(32 * r, 32 * c4))
                                nc.vector.tensor_reduce(
                                    out=minb2[:, col:col + 1, g],
                                    in_=ps,
                                    axis=mybir.AxisListType.X,
                                    op=MIN)
                            col += 1
                nc.vector.tensor_reduce(
                    out=minbuf, in_=minb2, axis=mybir.AxisListType.X, op=MIN)

            if repeat == 1:
                body()
            else:
                with tc.For_i(0, repeat, 1):
                    body()

            nc.sync.dma_start(out=mins_d, in_=minbuf)

    nc.compile()
    _CACHE[repeat] = nc
    return nc


def _hi_lo(a):
    """Split fp32 array into (hi, lo) bf16 parts, hi + lo ~= a."""
    hi = a.astype(BF)
    lo = (a - hi.astype(np.float32)).astype(BF)
    return hi, lo


def _forms(pts, nrm):
    """Stationary + moving operand forms for a point slab.

    pts: [nb, N, 3] f32, nrm: [nb, N] f32 -> (stat, mov) [nb, 13, N] bf16.
    """
    nb = pts.shape[0]
    h, l = _hi_lo(pts)                       # [nb, N, 3]
    nh = nrm.astype(BF)
    r1 = nrm - nh.astype(np.float32)
    nl = r1.astype(BF)
    nl2 = (r1 - nl.astype(np.float32)).astype(BF)
    m2h = (-2.0 * h.astype(np.float32)).astype(BF)
    m2l = (-2.0 * l.astype(np.float32)).astype(BF)
    one = np.ones((nb, N), BF)

    hT = np.moveaxis(h, 2, 1)                # [nb, 3, N]
    lT = np.moveaxis(l, 2, 1)
    m2hT = np.moveaxis(m2h, 2, 1)
    m2lT = np.moveaxis(m2l, 2, 1)

    # cross rows pair (h,h) (h,l) (l,h) (l,l); then 3 own-norm rows x ones,
    # then 3 ones x other-norm rows.
    stat = np.concatenate(
        [hT, hT, lT, lT,
         nh[:, None], nl[:, None], nl2[:, None],
         one[:, None], one[:, None], one[:, None]],
        axis=1)
    mov = np.concatenate(
        [m2hT, m2lT, m2hT, m2lT,
         one[:, None], one[:, None], one[:, None],
         nh[:, None], nl[:, None], nl2[:, None]],
        axis=1)
    return (np.ascontiguousarray(stat, dtype=BF),
            np.ascontiguousarray(mov, dtype=BF))


def _prep_inputs(array1, array2):
    x = np.asarray(array1, dtype=np.float32)
    y = np.asarray(array2, dtype=np.float32)
    nx = (x * x).sum(-1)                     # [B, N]
    ny = (y * y).sum(-1)
    in_maps = []
    for j in range(N_CORES):
        s = slice(2 * j, 2 * j + 2)
        xs, xm = _forms(x[s], nx[s])
        ys, ym = _forms(y[s], ny[s])
        in_maps.append({"xs": xs, "ym": ym, "ys": ys, "xm": xm})
    return in_maps


def _finish(results):
    total = 0.0
    for r in results:
        m = r["mins"].astype(np.float64)
        total += np.sqrt(np.maximum(m, 0.0)).sum()
    val = total / (B * N) / 2.0 * 1000.0
    return np.float32(val)


def kernel(array1, array2):
    from concourse import bass_utils
    nc = _build(repeat=1)
    in_maps = _prep_inputs(array1, array2)
    res = bass_utils.run_bass_kernel_spmd(nc, in_maps,
                                          core_ids=list(range(N_CORES)))
    return _finish(res.results)


# revision 9
# speedup vs baseline: 2034.2616x; 2034.2616x over previous
"""Chamfer distance (nn_L1_ChamferEval) Trainium2 Bass kernel.

Full inputs: array1 [16, 4096, 3] f32, array2 [16, 4096, 3] f32.
Output: scalar f32 = (mean(sqrt(min_m d)) + mean(sqrt(min_n d))) / 2 * 1000.

Strategy
--------
Data-parallel over batch: core j handles batches {2j, 2j+1}.

d[n,m] = ||x_n||^2 + ||y_m||^2 - 2<x_n,y_m> is computed on the tensor
engine as a single K=13 matmul using augmented rows, with fp32 values
split into bf16 hi/lo pairs so the bf16-rate PE path keeps ~fp32-product
precision (dropped lo*lo terms ~2^-18 relative):

  stationary rows: [xh0..2, xh0..2, xl0..2, nx_h, nx_l, 1, 1]
  moving rows:     [-2yh0..2, -2yl0..2, -2yh0..2, 1, 1, ny_h, ny_l]

K=13 < 32, so 16 independent 32x32 PE sub-tiles run concurrently via
tile_position (inferred from operand base partitions): row-strip r holds a
replica of the operands at partitions 32r, col-strip c produces n-block c.
One "burst" = 16 matmuls filling a [128 n, 2048 m] PSUM block (4 banks).

The min-reduction drains PSUM through both the DVE and ACT ports:
ACT copies the upper 1024 columns to SBUF while DVE runs a single
tensor_tensor_reduce(min, min) consuming (lower 1024 PSUM, ACT copy)
pairs — 2 values/cycle/lane with a chained per-partition running min.

Per-core device output: the 128 x (2 batch * 2 dir * 32 tiles) matrix of
min distances. Host finishes with relu/sqrt/mean in float64 (ordering of
the 131072 min values is irrelevant to the final mean).
"""
import sys
import numpy as np

sys.path.insert(0, "/opt/trn_rl_repo")

import ml_dtypes

N_CORES = 8
B, N, C = 16, 4096, 3
BPC = B // N_CORES          # batches per core
K = 24                      # augmented contraction rows
NT = N // 128               # 32 n-tiles per batch-direction
BF = ml_dtypes.bfloat16

_CACHE = {}


def _build(repeat=1):
    """Build + compile the 8-core bass program. Cached per `repeat`."""
    if repeat in _CACHE:
        return _CACHE[repeat]
    import concourse.bacc as bacc
    import concourse.tile as tile
    from concourse import mybir

    f32 = mybir.dt.float32
    bf16 = mybir.dt.bfloat16
    MIN = mybir.AluOpType.min

    nc = bacc.Bacc("TRN2", target_bir_lowering=False, debug=False,
                   enable_asserts=True, num_devices=N_CORES)

    xs_d = nc.dram_tensor("xs", [BPC, K, N], bf16, kind="ExternalInput").ap()
    ym_d = nc.dram_tensor("ym", [BPC, K, N], bf16, kind="ExternalInput").ap()
    ys_d = nc.dram_tensor("ys", [BPC, K, N], bf16, kind="ExternalInput").ap()
    xm_d = nc.dram_tensor("xm", [BPC, K, N], bf16, kind="ExternalInput").ap()
    mins_d = nc.dram_tensor("mins", [128, 4 * NT], f32,
                            kind="ExternalOutput").ap()

    with tile.TileContext(nc) as tc:
        with (
            tc.tile_pool(name="forms", bufs=1) as formp,
            tc.tile_pool(name="minb", bufs=1) as minbp,
            tc.tile_pool(name="psum", bufs=2, space="PSUM") as psp,
        ):
            # Load the 4 operand forms x 2 batches, replicated on the 4
            # 32-partition row strips the PE sub-tiles read from.
            sb = {}
            for name, dram in (("xs", xs_d), ("ym", ym_d),
                               ("ys", ys_d), ("xm", xm_d)):
                for b in range(BPC):
                    t = formp.tile([128, N], bf16, tag=f"{name}{b}")
                    nc.vector.memset(t, 0.0)
                    for s in range(4):
                        nc.sync.dma_start(out=t[32 * s:32 * s + K, :],
                                          in_=dram[b])
                    sb[(name, b)] = t

            minb2 = minbp.tile([128, 4 * NT, 2], f32)
            minbuf = minbp.tile([128, 4 * NT], f32)

            def body():
                col = 0
                for b in range(BPC):
                    for d in range(2):
                        stat = sb[("xs", b)] if d == 0 else sb[("ys", b)]
                        mov = sb[("ym", b)] if d == 0 else sb[("xm", b)]
                        for t in range(NT):
                            for g in range(2):
                                ps = psp.tile([128, 2048], f32)
                                for r in range(4):
                                    for c4 in range(4):
                                        nc.tensor.matmul(
                                            out=ps[32 * c4:32 * c4 + 32,
                                                   512 * r:512 * (r + 1)],
                                            lhsT=stat[32 * r:32 * r + K,
                                                      128 * t + 32 * c4:
                                                      128 * t + 32 * c4 + 32],
                                            rhs=mov[32 * r:32 * r + K,
                                                    2048 * g + 512 * r:
                                                    2048 * g + 512 * (r + 1)],
                                            start=True, stop=True,
                                            tile_position=(32 * r, 32 * c4))
                                nc.vector.tensor_reduce(
                                    out=minb2[:, col:col + 1, g],
                                    in_=ps,
                                    axis=mybir.AxisListType.X,
                                    op=MIN)
                            col += 1
                nc.vector.tensor_reduce(
                    out=minbuf, in_=minb2, axis=mybir.AxisListType.X, op=MIN)

            if repeat == 1:
                body()
            else:
                with tc.For_i(0, repeat, 1):
                    body()

            nc.sync.dma_start(out=mins_d, in_=minbuf)

    nc.compile()
    _CACHE[repeat] = nc
    return nc


def _hi_lo(a):
    """Split fp32 array into (hi, lo) bf16 parts, hi + lo ~= a."""
    hi = a.astype(BF)
    lo = (a - hi.astype(np.float32)).astype(BF)
    return hi, lo


def _forms(pts, nrm):
    """Stationary + moving operand forms for a point slab.

    pts: [nb, N, 3] f32, nrm: [nb, N] f32 -> (stat, mov) [nb, 13, N] bf16.
    """
    nb = pts.shape[0]
    # 3-level bf16 split of coordinates: x ~ x1 + x2 + x3 (residual 2^-28)
    x1 = pts.astype(BF)
    r = pts - x1.astype(np.float32)
    x2 = r.astype(BF)
    x3 = (r - x2.astype(np.float32)).astype(BF)
    nh = nrm.astype(BF)
    r1 = nrm - nh.astype(np.float32)
    nl = r1.astype(BF)
    nl2 = (r1 - nl.astype(np.float32)).astype(BF)
    m1 = (-2.0 * x1.astype(np.float32)).astype(BF)   # exact
    m2 = (-2.0 * x2.astype(np.float32)).astype(BF)
    m3 = (-2.0 * x3.astype(np.float32)).astype(BF)
    one = np.ones((nb, N), BF)

    t1 = np.moveaxis(x1, 2, 1)               # [nb, 3, N]
    t2 = np.moveaxis(x2, 2, 1)
    t3 = np.moveaxis(x3, 2, 1)
    mt1 = np.moveaxis(m1, 2, 1)
    mt2 = np.moveaxis(m2, 2, 1)
    mt3 = np.moveaxis(m3, 2, 1)

    # cross-product classes (1,1)(1,2)(2,1)(1,3)(3,1)(2,2); dropped terms
    # are O(2^-27 |x||y|). Then 3 own-norm rows x ones, 3 ones x other-norm.
    stat = np.concatenate(
        [t1, t1, t2, t1, t3, t2,
         nh[:, None], nl[:, None], nl2[:, None],
         one[:, None], one[:, None], one[:, None]],
        axis=1)
    mov = np.concatenate(
        [mt1, mt2, mt1, mt3, mt1, mt2,
         one[:, None], one[:, None], one[:, None],
         nh[:, None], nl[:, None], nl2[:, None]],
        axis=1)
    return (np.ascontiguousarray(stat, dtype=BF),
            np.ascontiguousarray(mov, dtype=BF))


def _prep_inputs(array1, array2):
    x = np.asarray(array1, dtype=np.float32)
    y = np.asarray(array2, dtype=np.float32)
    nx = (x * x).sum(-1)                     # [B, N]
    ny = (y * y).sum(-1)
    in_maps = []
    for j in range(N_CORES):
        s = slice(2 * j, 2 * j + 2)
        xs, xm = _forms(x[s], nx[s])
        ys, ym = _forms(y[s], ny[s])
        in_maps.append({"xs": xs, "ym": ym, "ys": ys, "xm": xm})
    return in_maps


def _finish(results):
    total = 0.0
    for r in results:
        m = r["mins"].astype(np.float64)
        total += np.sqrt(np.maximum(m, 0.0)).sum()
    val = total / (B * N) / 2.0 * 1000.0
    return np.float32(val)


def kernel(array1, array2):
    from concourse import bass_utils
    nc = _build(repeat=1)
    in_maps = _prep_inputs(array1, array2)
    res = bass_utils.run_bass_kernel_spmd(nc, in_maps,
                                          core_ids=list(range(N_CORES)))
    return _finish(res.results)


# revision 10
# speedup vs baseline: 3008.8927x; 1.4791x over previous
"""Chamfer distance (nn_L1_ChamferEval) Trainium2 Bass kernel.

Full inputs: array1 [16, 4096, 3] f32, array2 [16, 4096, 3] f32.
Output: scalar f32 = (mean(sqrt(min_m d)) + mean(sqrt(min_n d))) / 2 * 1000.

Strategy
--------
Data-parallel over batch: core j handles batches {2j, 2j+1}.

d[n,m] = ||x_n||^2 + ||y_m||^2 - 2<x_n,y_m> is computed on the tensor
engine as a single K=13 matmul using augmented rows, with fp32 values
split into bf16 hi/lo pairs so the bf16-rate PE path keeps ~fp32-product
precision (dropped lo*lo terms ~2^-18 relative):

  stationary rows: [xh0..2, xh0..2, xl0..2, nx_h, nx_l, 1, 1]
  moving rows:     [-2yh0..2, -2yl0..2, -2yh0..2, 1, 1, ny_h, ny_l]

K=13 < 32, so 16 independent 32x32 PE sub-tiles run concurrently via
tile_position (inferred from operand base partitions): row-strip r holds a
replica of the operands at partitions 32r, col-strip c produces n-block c.
One "burst" = 16 matmuls filling a [128 n, 2048 m] PSUM block (4 banks).

The min-reduction drains PSUM through both the DVE and ACT ports:
ACT copies the upper 1024 columns to SBUF while DVE runs a single
tensor_tensor_reduce(min, min) consuming (lower 1024 PSUM, ACT copy)
pairs — 2 values/cycle/lane with a chained per-partition running min.

Per-core device output: the 128 x (2 batch * 2 dir * 32 tiles) matrix of
min distances. Host finishes with relu/sqrt/mean in float64 (ordering of
the 131072 min values is irrelevant to the final mean).
"""
import sys
import numpy as np

sys.path.insert(0, "/opt/trn_rl_repo")

import ml_dtypes

N_CORES = 8
B, N, C = 16, 4096, 3
BPC = B // N_CORES          # batches per core
K = 24                      # augmented contraction rows
NT = N // 128               # 32 n-tiles per batch-direction
BF = ml_dtypes.bfloat16

_CACHE = {}


def _build(repeat=1):
    """Build + compile the 8-core bass program. Cached per `repeat`."""
    if repeat in _CACHE:
        return _CACHE[repeat]
    import concourse.bacc as bacc
    import concourse.tile as tile
    from concourse import mybir

    f32 = mybir.dt.float32
    bf16 = mybir.dt.bfloat16
    MIN = mybir.AluOpType.min

    nc = bacc.Bacc("TRN2", target_bir_lowering=False, debug=False,
                   enable_asserts=True, num_devices=N_CORES)

    xs_d = nc.dram_tensor("xs", [BPC, K, N], bf16, kind="ExternalInput").ap()
    ym_d = nc.dram_tensor("ym", [BPC, K, N], bf16, kind="ExternalInput").ap()
    ys_d = nc.dram_tensor("ys", [BPC, K, N], bf16, kind="ExternalInput").ap()
    xm_d = nc.dram_tensor("xm", [BPC, K, N], bf16, kind="ExternalInput").ap()
    mins_d = nc.dram_tensor("mins", [128, 4 * NT], f32,
                            kind="ExternalOutput").ap()

    with tile.TileContext(nc) as tc:
        with (
            tc.tile_pool(name="forms", bufs=1) as formp,
            tc.tile_pool(name="minb", bufs=1) as minbp,
            tc.tile_pool(name="psum", bufs=2, space="PSUM") as psp,
        ):
            # Load the 4 operand forms x 2 batches, replicated on the 4
            # 32-partition row strips the PE sub-tiles read from.
            sb = {}
            for name, dram in (("xs", xs_d), ("ym", ym_d),
                               ("ys", ys_d), ("xm", xm_d)):
                for b in range(BPC):
                    t = formp.tile([128, N], bf16, tag=f"{name}{b}")
                    nc.vector.memset(t, 0.0)
                    for s in range(4):
                        nc.sync.dma_start(out=t[32 * s:32 * s + K, :],
                                          in_=dram[b])
                    sb[(name, b)] = t

            minb2 = minbp.tile([128, 4 * NT, 2], f32)
            minbuf = minbp.tile([128, 4 * NT], f32)

            def body():
                col = 0
                for b in range(BPC):
                    for d in range(2):
                        stat = sb[("xs", b)] if d == 0 else sb[("ys", b)]
                        mov = sb[("ym", b)] if d == 0 else sb[("xm", b)]
                        for t in range(NT):
                            for g in range(2):
                                ps = psp.tile([128, 2048], f32)
                                for r in range(4):
                                    nc.tensor.matmul(
                                        out=ps[:, 512 * r:512 * (r + 1)],
                                        lhsT=stat[0:K,
                                                  128 * t:128 * (t + 1)],
                                        rhs=mov[0:K,
                                                2048 * g + 512 * r:
                                                2048 * g + 512 * (r + 1)],
                                        start=True, stop=True)
                                nc.vector.tensor_reduce(
                                    out=minb2[:, col:col + 1, g],
                                    in_=ps,
                                    axis=mybir.AxisListType.X,
                                    op=MIN)
                            col += 1
                nc.vector.tensor_reduce(
                    out=minbuf, in_=minb2, axis=mybir.AxisListType.X, op=MIN)

            if repeat == 1:
                body()
            else:
                with tc.For_i(0, repeat, 1):
                    body()

            nc.sync.dma_start(out=mins_d, in_=minbuf)

    nc.compile()
    _CACHE[repeat] = nc
    return nc


def _hi_lo(a):
    """Split fp32 array into (hi, lo) bf16 parts, hi + lo ~= a."""
    hi = a.astype(BF)
    lo = (a - hi.astype(np.float32)).astype(BF)
    return hi, lo


def _forms(pts, nrm):
    """Stationary + moving operand forms for a point slab.

    pts: [nb, N, 3] f32, nrm: [nb, N] f32 -> (stat, mov) [nb, 13, N] bf16.
    """
    nb = pts.shape[0]
    # 3-level bf16 split of coordinates: x ~ x1 + x2 + x3 (residual 2^-28)
    x1 = pts.astype(BF)
    r = pts - x1.astype(np.float32)
    x2 = r.astype(BF)
    x3 = (r - x2.astype(np.float32)).astype(BF)
    nh = nrm.astype(BF)
    r1 = nrm - nh.astype(np.float32)
    nl = r1.astype(BF)
    nl2 = (r1 - nl.astype(np.float32)).astype(BF)
    m1 = (-2.0 * x1.astype(np.float32)).astype(BF)   # exact
    m2 = (-2.0 * x2.astype(np.float32)).astype(BF)
    m3 = (-2.0 * x3.astype(np.float32)).astype(BF)
    one = np.ones((nb, N), BF)

    t1 = np.moveaxis(x1, 2, 1)               # [nb, 3, N]
    t2 = np.moveaxis(x2, 2, 1)
    t3 = np.moveaxis(x3, 2, 1)
    mt1 = np.moveaxis(m1, 2, 1)
    mt2 = np.moveaxis(m2, 2, 1)
    mt3 = np.moveaxis(m3, 2, 1)

    # cross-product classes (1,1)(1,2)(2,1)(1,3)(3,1)(2,2); dropped terms
    # are O(2^-27 |x||y|). Then 3 own-norm rows x ones, 3 ones x other-norm.
    stat = np.concatenate(
        [t1, t1, t2, t1, t3, t2,
         nh[:, None], nl[:, None], nl2[:, None],
         one[:, None], one[:, None], one[:, None]],
        axis=1)
    mov = np.concatenate(
        [mt1, mt2, mt1, mt3, mt1, mt2,
         one[:, None], one[:, None], one[:, None],
         nh[:, None], nl[:, None], nl2[:, None]],
        axis=1)
    return (np.ascontiguousarray(stat, dtype=BF),
            np.ascontiguousarray(mov, dtype=BF))


def _prep_inputs(array1, array2):
    x = np.asarray(array1, dtype=np.float32)
    y = np.asarray(array2, dtype=np.float32)
    nx = (x * x).sum(-1)                     # [B, N]
    ny = (y * y).sum(-1)
    in_maps = []
    for j in range(N_CORES):
        s = slice(2 * j, 2 * j + 2)
        xs, xm = _forms(x[s], nx[s])
        ys, ym = _forms(y[s], ny[s])
        in_maps.append({"xs": xs, "ym": ym, "ys": ys, "xm": xm})
    return in_maps


def _finish(results):
    total = 0.0
    for r in results:
        m = r["mins"].astype(np.float64)
        total += np.sqrt(np.maximum(m, 0.0)).sum()
    val = total / (B * N) / 2.0 * 1000.0
    return np.float32(val)


def kernel(array1, array2):
    from concourse import bass_utils
    nc = _build(repeat=1)
    in_maps = _prep_inputs(array1, array2)
    res = bass_utils.run_bass_kernel_spmd(nc, in_maps,
                                          core_ids=list(range(N_CORES)))
    return _finish(res.results)


# revision 11
# speedup vs baseline: 3836.0333x; 1.2749x over previous
"""Chamfer distance (nn_L1_ChamferEval) Trainium2 Bass kernel.

Full inputs: array1 [16, 4096, 3] f32, array2 [16, 4096, 3] f32.
Output: scalar f32 = (mean(sqrt(min_m d)) + mean(sqrt(min_n d))) / 2 * 1000.

Strategy
--------
Data-parallel over batch: core j handles batches {2j, 2j+1}.

d[n,m] = ||x_n||^2 + ||y_m||^2 - 2<x_n,y_m> is computed on the tensor
engine as a single K=24 matmul using augmented rows, with fp32 values
split into a 3-level bf16 sum (x ~ x1+x2+x3, residual 2^-28) so the
bf16-rate PE path keeps fp32-product precision. The 6 significant
cross-product classes (1,1)(1,2)(2,1)(1,3)(3,1)(2,2) plus 3-row hi/lo
splits of each norm give |d_err| ~ 2e-7 — required because the min
squared distances here are tiny (median ~7e-5) and sqrt amplifies any
d-noise into a systematic (Jensen) bias of the final mean.

One "burst" = 4 matmuls (lhsT [24,128] stationary, rhs [24,512] moving)
filling a [128 n, 2048 m] PSUM block (4 banks); two such PSUM tiles
double-buffer against the reducer. The min-reduction is a single DVE
tensor_reduce(min) per burst over the whole 4-bank block; burst pairs
land in minb2[:, col, g] and one last reduce folds g.

Note: the unwritten partition rows K..31 of the operand tiles must be
zeroed (memset) — the PE streams the full 32-row strip and stale SBUF
garbage otherwise corrupts d.

Per-core device output: the 128 x (2 batch * 2 dir * 32 tiles) matrix of
min distances. Host finishes with relu/sqrt/mean in float64 (ordering of
the 131072 min values is irrelevant to the final mean).
"""
import sys
import numpy as np

sys.path.insert(0, "/opt/trn_rl_repo")

import ml_dtypes

N_CORES = 8
B, N, C = 16, 4096, 3
BPC = B // N_CORES          # batches per core
K = 24                      # augmented contraction rows
NT = N // 128               # 32 n-tiles per batch-direction
BF = ml_dtypes.bfloat16

_CACHE = {}


def _build(repeat=1):
    """Build + compile the 8-core bass program. Cached per `repeat`."""
    if repeat in _CACHE:
        return _CACHE[repeat]
    import concourse.bacc as bacc
    import concourse.tile as tile
    from concourse import mybir

    f32 = mybir.dt.float32
    bf16 = mybir.dt.bfloat16
    MIN = mybir.AluOpType.min

    nc = bacc.Bacc("TRN2", target_bir_lowering=False, debug=False,
                   enable_asserts=True, num_devices=N_CORES)

    xs_d = nc.dram_tensor("xs", [BPC, K, N], bf16, kind="ExternalInput").ap()
    ym_d = nc.dram_tensor("ym", [BPC, K, N], bf16, kind="ExternalInput").ap()
    ys_d = nc.dram_tensor("ys", [BPC, K, N], bf16, kind="ExternalInput").ap()
    xm_d = nc.dram_tensor("xm", [BPC, K, N], bf16, kind="ExternalInput").ap()
    mins_d = nc.dram_tensor("mins", [128, 4 * NT], f32,
                            kind="ExternalOutput").ap()

    with tile.TileContext(nc) as tc:
        with (
            tc.tile_pool(name="forms", bufs=1) as formp,
            tc.tile_pool(name="minb", bufs=1) as minbp,
            tc.tile_pool(name="psum", bufs=2, space="PSUM") as psp,
        ):
            # Load the 4 operand forms x 2 batches, replicated on the 4
            # 32-partition row strips the PE sub-tiles read from.
            sb = {}
            for name, dram in (("xs", xs_d), ("ym", ym_d),
                               ("ys", ys_d), ("xm", xm_d)):
                for b in range(BPC):
                    t = formp.tile([128, N], bf16, tag=f"{name}{b}")
                    nc.vector.memset(t, 0.0)
                    for s in range(4):
                        nc.sync.dma_start(out=t[32 * s:32 * s + K, :],
                                          in_=dram[b])
                    sb[(name, b)] = t

            minb2 = minbp.tile([128, 4 * NT, 2], f32)
            minbuf = minbp.tile([128, 4 * NT], f32)

            def body():
                col = 0
                for b in range(BPC):
                    for d in range(2):
                        stat = sb[("xs", b)] if d == 0 else sb[("ys", b)]
                        mov = sb[("ym", b)] if d == 0 else sb[("xm", b)]
                        for t in range(NT):
                            for g in range(2):
                                ps = psp.tile([128, 2048], f32)
                                for r in range(4):
                                    nc.tensor.matmul(
                                        out=ps[:, 512 * r:512 * (r + 1)],
                                        lhsT=stat[0:K,
                                                  128 * t:128 * (t + 1)],
                                        rhs=mov[0:K,
                                                2048 * g + 512 * r:
                                                2048 * g + 512 * (r + 1)],
                                        start=True, stop=True)
                                nc.vector.tensor_reduce(
                                    out=minb2[:, col:col + 1, g],
                                    in_=ps,
                                    axis=mybir.AxisListType.X,
                                    op=MIN)
                            col += 1
                nc.vector.tensor_reduce(
                    out=minbuf, in_=minb2, axis=mybir.AxisListType.X, op=MIN)

            if repeat == 1:
                body()
            else:
                with tc.For_i(0, repeat, 1):
                    body()

            nc.sync.dma_start(out=mins_d, in_=minbuf)

    nc.compile()
    _CACHE[repeat] = nc
    return nc


def _hi_lo(a):
    """Split fp32 array into (hi, lo) bf16 parts, hi + lo ~= a."""
    hi = a.astype(BF)
    lo = (a - hi.astype(np.float32)).astype(BF)
    return hi, lo


def _forms(pts, nrm):
    """Stationary + moving operand forms for a point slab.

    pts: [nb, N, 3] f32, nrm: [nb, N] f32 -> (stat, mov) [nb, 13, N] bf16.
    """
    nb = pts.shape[0]
    # 3-level bf16 split of coordinates: x ~ x1 + x2 + x3 (residual 2^-28)
    x1 = pts.astype(BF)
    r = pts - x1.astype(np.float32)
    x2 = r.astype(BF)
    x3 = (r - x2.astype(np.float32)).astype(BF)
    nh = nrm.astype(BF)
    r1 = nrm - nh.astype(np.float32)
    nl = r1.astype(BF)
    nl2 = (r1 - nl.astype(np.float32)).astype(BF)
    m1 = (-2.0 * x1.astype(np.float32)).astype(BF)   # exact
    m2 = (-2.0 * x2.astype(np.float32)).astype(BF)
    m3 = (-2.0 * x3.astype(np.float32)).astype(BF)
    one = np.ones((nb, N), BF)

    t1 = np.moveaxis(x1, 2, 1)               # [nb, 3, N]
    t2 = np.moveaxis(x2, 2, 1)
    t3 = np.moveaxis(x3, 2, 1)
    mt1 = np.moveaxis(m1, 2, 1)
    mt2 = np.moveaxis(m2, 2, 1)
    mt3 = np.moveaxis(m3, 2, 1)

    # cross-product classes (1,1)(1,2)(2,1)(1,3)(3,1)(2,2); dropped terms
    # are O(2^-27 |x||y|). Then 3 own-norm rows x ones, 3 ones x other-norm.
    stat = np.concatenate(
        [t1, t1, t2, t1, t3, t2,
         nh[:, None], nl[:, None], nl2[:, None],
         one[:, None], one[:, None], one[:, None]],
        axis=1)
    mov = np.concatenate(
        [mt1, mt2, mt1, mt3, mt1, mt2,
         one[:, None], one[:, None], one[:, None],
         nh[:, None], nl[:, None], nl2[:, None]],
        axis=1)
    return (np.ascontiguousarray(stat, dtype=BF),
            np.ascontiguousarray(mov, dtype=BF))


def _prep_inputs(array1, array2):
    x = np.asarray(array1, dtype=np.float32)
    y = np.asarray(array2, dtype=np.float32)
    nx = (x * x).sum(-1)                     # [B, N]
    ny = (y * y).sum(-1)
    in_maps = []
    for j in range(N_CORES):
        s = slice(2 * j, 2 * j + 2)
        xs, xm = _forms(x[s], nx[s])
        ys, ym = _forms(y[s], ny[s])
        in_maps.append({"xs": xs, "ym": ym, "ys": ys, "xm": xm})
    return in_maps


def _finish(results):
    total = 0.0
    for r in results:
        m = r["mins"].astype(np.float64)
        total += np.sqrt(np.maximum(m, 0.0)).sum()
    val = total / (B * N) / 2.0 * 1000.0
    return np.float32(val)


def kernel(array1, array2):
    from concourse import bass_utils
    nc = _build(repeat=1)
    in_maps = _prep_inputs(array1, array2)
    res = bass_utils.run_bass_kernel_spmd(nc, in_maps,
                                          core_ids=list(range(N_CORES)))
    return _finish(res.results)
